# revision 1
# baseline (speedup 1.0000x reference)
"""3-layer GCN (message passing) on 8 Trainium2 NeuronCores.

Strategy: shard nodes (and their incoming edges, grouped by dst tile) across
the 8 cores.  Per layer:
  - each core densely transforms its own node shard (PE matmul),
  - AllGather replicates the transformed feature table to every core,
  - per 128-dst tile, dma_gather pulls h[src] rows (HBM -> SBUF), DVE scales
    them by the GCN norm, and PE matmuls against an on-device-built one-hot
    dst mask perform the segment-sum directly in PSUM (bias preloaded).
ReLU on ACT, mean-pool via PE matmuls with one-hot graph masks, a tiny
AllReduce, and log_softmax on device.  Host work is limited to static graph
preprocessing (norm / index layout).
"""

import os
import sys
from contextlib import ExitStack

import numpy as np

sys.path.insert(0, "/opt/trn_rl_repo")

# problem constants (hardcoded per the harness contract)
N = 50000
E = 800000
F = 128          # DIN == HID == 128
OUTC = 10
G = 64
NCORES = 8
NPC = N // NCORES            # 6250 nodes per core
TILES = 49                   # ceil(6250/128)
NPAD = TILES * 128           # 6272
ROWS = NPAD * NCORES         # 50176 table rows
HALF = NPAD * (NCORES // 2)  # 25088  (< 32768 so int16 indices work)

_COMPILED = None  # (nc, SEC) cache


def _build_program(SEC):
    """SEC = subchunks (of 128 edges) per lo/hi section of one dst tile."""
    import concourse.tile as tile
    from concourse import bacc, mybir

    f32 = mybir.dt.float32
    i32 = mybir.dt.int32
    i16 = mybir.dt.int16
    AF = mybir.ActivationFunctionType
    ALU = mybir.AluOpType
    SUBS = 2 * SEC           # subchunks per tile
    SECE = SEC * 128         # edges per section
    SECI = SECE // 16        # idx columns per section

    nc = bacc.Bacc("TRN2", target_bir_lowering=False, debug=False,
                   enable_asserts=False, num_devices=NCORES,
                   num_swdge_queues=4, dynamic_dma_scratch_size=65536)

    # ---- I/O tensors ----
    x_ownT = nc.dram_tensor("x_ownT", [F, NPAD], f32, kind="ExternalInput")
    W1 = nc.dram_tensor("W1", [F, F], f32, kind="ExternalInput")
    W2 = nc.dram_tensor("W2", [F, F], f32, kind="ExternalInput")
    W3 = nc.dram_tensor("W3", [F, OUTC], f32, kind="ExternalInput")
    b1rep = nc.dram_tensor("b1rep", [128, F], f32, kind="ExternalInput")
    b2rep = nc.dram_tensor("b2rep", [128, F], f32, kind="ExternalInput")
    b3rep = nc.dram_tensor("b3rep", [G, OUTC], f32, kind="ExternalInput")
    eye = nc.dram_tensor("eye", [128, 128], f32, kind="ExternalInput")
    iota_g = nc.dram_tensor("iota_g", [128, G], i32, kind="ExternalInput")
    iota_d = nc.dram_tensor("iota_d", [128, 128], f32, kind="ExternalInput")
    batch_t = nc.dram_tensor("batch_t", [128, TILES], i32, kind="ExternalInput")
    src_idx = nc.dram_tensor("src_idx", [128, TILES * 2 * SECI], i16,
                             kind="ExternalInput")
    dstf = nc.dram_tensor("dstf", [128, TILES * SUBS], f32,
                          kind="ExternalInput")
    normv = nc.dram_tensor("normv", [128, TILES * SUBS], f32,
                           kind="ExternalInput")
    out_d = nc.dram_tensor("out", [G, OUTC], f32, kind="ExternalOutput")

    # ---- internal DRAM ----
    T_own = [nc.dram_tensor(f"T{l}_own", [NPAD, F], f32) for l in range(3)]
    T_full = [nc.dram_tensor(f"T{l}_full", [ROWS, F], f32) for l in range(3)]
    pool_in = nc.dram_tensor("pool_in", [G, OUTC + 1], f32)
    pool_out = nc.dram_tensor("pool_out", [G, OUTC + 1], f32)
    groups = [list(range(NCORES))]

    with tile.TileContext(nc) as tc, ExitStack() as ctx:
        const = ctx.enter_context(tc.tile_pool(name="const", bufs=1))
        sb = ctx.enter_context(tc.tile_pool(name="sb", bufs=3))
        gpool = ctx.enter_context(tc.tile_pool(name="gath", bufs=4))
        idxp = ctx.enter_context(tc.tile_pool(name="idx", bufs=4))
        psum = ctx.enter_context(tc.tile_pool(name="psum", bufs=2, space="PSUM"))

        def load_const(tag, dram, shape, dtype=f32):
            t = const.tile(shape, dtype, tag=tag)
            nc.sync.dma_start(t[:], dram.ap())
            return t

        W1_sb = load_const("W1", W1, [F, F])
        W2_sb = load_const("W2", W2, [F, F])
        W3_sb = load_const("W3", W3, [F, OUTC])
        b1_sb = load_const("b1", b1rep, [128, F])
        b2_sb = load_const("b2", b2rep, [128, F])
        b3_sb = load_const("b3", b3rep, [G, OUTC])
        eye_sb = load_const("eye", eye, [128, 128])
        iota_sb = load_const("iota", iota_g, [128, G], i32)
        iotad_sb = load_const("iotad", iota_d, [128, 128])
        batch_sb = load_const("batch", batch_t, [128, TILES], i32)
        xT_sb = load_const("xT", x_ownT, [F, NPAD])

        def rng(t):
            return slice(t * 128, (t + 1) * 128)

        # ---- phase A: T0 = x_own @ W1, write own shard, allgather ----
        for t in range(TILES):
            ps = psum.tile([128, F], f32, tag="tps")
            nc.tensor.matmul(ps[:], xT_sb[:, rng(t)], W1_sb[:],
                             start=True, stop=True)
            tt = sb.tile([128, F], f32, tag="tout")
            nc.scalar.activation(tt[:], ps[:], AF.Copy)
            nc.sync.dma_start(T_own[0].ap()[rng(t), :], tt[:])
        nc.gpsimd.collective_compute(
            "AllGather", ALU.bypass, replica_groups=groups,
            ins=[T_own[0].ap().opt()], outs=[T_full[0].ap().opt()])

        # ---- aggregation of one dst tile into a PSUM tile ----
        GCAP = 8  # max subchunks (1024 descriptors) per dma_gather

        def agg_tile(l, t, bias_sb):
            g = gpool.tile([128, SUBS, F], f32, tag="g")
            for h in range(2):
                off = (2 * t + h) * SECI
                for p in range(0, SEC, GCAP):
                    take = min(GCAP, SEC - p)
                    si = idxp.tile([128, take * 8], i16, tag="si")
                    nc.sync.dma_start(
                        si[:], src_idx.ap()[:, off + p * 8:off + (p + take) * 8])
                    nc.gpsimd.dma_gather(
                        g[:, h * SEC + p:h * SEC + p + take, :],
                        T_full[l].ap()[h * HALF:(h + 1) * HALF, :],
                        si[:], take * 128, take * 128, F,
                        queue_num=(2 * t + h) % 4)
            nv = idxp.tile([128, SUBS], f32, tag="nv")
            nc.sync.dma_start(nv[:], normv.ap()[:, t * SUBS:(t + 1) * SUBS])
            nc.vector.tensor_mul(
                g[:], g[:], nv[:].unsqueeze(2).broadcast_to([128, SUBS, F]))
            df = idxp.tile([128, SUBS], f32, tag="df")
            nc.sync.dma_start(df[:], dstf.ap()[:, t * SUBS:(t + 1) * SUBS])
            mk = gpool.tile([128, SUBS, 128], f32, tag="mk")
            nc.vector.tensor_tensor(
                mk[:], df[:].unsqueeze(2).broadcast_to([128, SUBS, 128]),
                iotad_sb[:].unsqueeze(1).broadcast_to([128, SUBS, 128]),
                op=ALU.is_equal)
            ps = psum.tile([128, F], f32, tag="aps")
            if bias_sb is None:
                for s in range(SUBS):
                    nc.tensor.matmul(ps[:], mk[:, s, :], g[:, s, :],
                                     start=(s == 0), stop=(s == SUBS - 1))
            else:
                nc.vector.tensor_copy(ps[:], bias_sb[:])
                for s in range(SUBS):
                    nc.tensor.matmul(ps[:], mk[:, s, :], g[:, s, :],
                                     start=False, stop=(s == SUBS - 1),
                                     skip_group_check=True)
            return ps

        # ---- layer 1: T1 = relu(agg + b1) @ W2 ----
        for t in range(TILES):
            ps = agg_tile(0, t, b1_sb)
            h = sb.tile([128, F], f32, tag="h")
            nc.scalar.activation(h[:], ps[:], AF.Relu)
            hT = psum.tile([128, 128], f32, tag="tr")
            nc.tensor.transpose(hT[:], h[:], eye_sb[:])
            s2 = sb.tile([128, 128], f32, tag="s2")
            nc.scalar.activation(s2[:], hT[:], AF.Copy)
            ps2 = psum.tile([128, F], f32, tag="tps")
            nc.tensor.matmul(ps2[:], s2[:], W2_sb[:], start=True, stop=True)
            tt = sb.tile([128, F], f32, tag="tout")
            nc.scalar.activation(tt[:], ps2[:], AF.Copy)
            nc.sync.dma_start(T_own[1].ap()[rng(t), :], tt[:])
        nc.gpsimd.collective_compute(
            "AllGather", ALU.bypass, replica_groups=groups,
            ins=[T_own[1].ap().opt()], outs=[T_full[1].ap().opt()])

        # ---- layer 2: T2 = relu(agg + b2)   (W3 postponed: A(HW)=(AH)W) ----
        for t in range(TILES):
            ps = agg_tile(1, t, b2_sb)
            tt = sb.tile([128, F], f32, tag="tout")
            nc.scalar.activation(tt[:], ps[:], AF.Relu)
            nc.sync.dma_start(T_own[2].ap()[rng(t), :], tt[:])
        nc.gpsimd.collective_compute(
            "AllGather", ALU.bypass, replica_groups=groups,
            ins=[T_own[2].ap().opt()], outs=[T_full[2].ap().opt()])

        # ---- layer 3 + pooling ----
        pooled = const.tile([G, OUTC + 1], f32)
        nc.vector.memset(pooled[:], 0.0)
        for t in range(TILES):
            ps = agg_tile(2, t, None)
            a3 = sb.tile([128, F], f32, tag="h")
            nc.scalar.activation(a3[:], ps[:], AF.Copy)
            aT = psum.tile([128, 128], f32, tag="tr")
            nc.tensor.transpose(aT[:], a3[:], eye_sb[:])
            s2 = sb.tile([128, 128], f32, tag="s2")
            nc.scalar.activation(s2[:], aT[:], AF.Copy)
            op = psum.tile([128, OUTC], f32, tag="tps")
            nc.tensor.matmul(op[:], s2[:], W3_sb[:], start=True, stop=True)
            re = sb.tile([128, OUTC + 1], f32, tag="re")
            nc.scalar.activation(re[:, 0:OUTC], op[:], AF.Copy)
            nc.vector.memset(re[:, OUTC:OUTC + 1], 1.0)
            pm = sb.tile([128, G], f32, tag="pm")
            nc.vector.tensor_tensor(
                pm[:], batch_sb[:, t:t + 1].broadcast_to([128, G]),
                iota_sb[:], op=ALU.is_equal)
            pp = psum.tile([G, OUTC + 1], f32, tag="pps")
            nc.tensor.matmul(pp[:], pm[:], re[:], start=True, stop=True)
            pc = sb.tile([G, OUTC + 1], f32, tag="pc")
            nc.scalar.activation(pc[:], pp[:], AF.Copy)
            nc.vector.tensor_add(pooled[:], pooled[:], pc[:])

        nc.sync.dma_start(pool_in.ap(), pooled[:])
        nc.gpsimd.collective_compute(
            "AllReduce", ALU.add, replica_groups=groups,
            ins=[pool_in.ap().opt()], outs=[pool_out.ap().opt()])
        pr = sb.tile([G, OUTC + 1], f32, tag="pr")
        nc.sync.dma_start(pr[:], pool_out.ap())

        cm = sb.tile([G, 1], f32, tag="cm")
        nc.vector.tensor_scalar_max(cm[:], pr[:, OUTC:OUTC + 1], 1.0)
        rec = sb.tile([G, 1], f32, tag="rec")
        nc.vector.reciprocal(rec[:], cm[:])
        m = sb.tile([G, OUTC], f32, tag="m")
        nc.vector.tensor_mul(m[:], pr[:, 0:OUTC],
                             rec[:].broadcast_to([G, OUTC]))
        nc.vector.tensor_add(m[:], m[:], b3_sb[:])
        mx = sb.tile([G, 1], f32, tag="mx")
        nc.vector.tensor_reduce(mx[:], m[:], mybir.AxisListType.XYZW, ALU.max)
        sh = sb.tile([G, OUTC], f32, tag="sh")
        nc.vector.tensor_sub(sh[:], m[:], mx[:].broadcast_to([G, OUTC]))
        ex = sb.tile([G, OUTC], f32, tag="ex")
        nc.scalar.activation(ex[:], sh[:], AF.Exp)
        sm = sb.tile([G, 1], f32, tag="sm")
        nc.vector.tensor_reduce(sm[:], ex[:], mybir.AxisListType.XYZW, ALU.add)
        ln = sb.tile([G, 1], f32, tag="ln")
        nc.scalar.activation(ln[:], sm[:], AF.Ln)
        res = sb.tile([G, OUTC], f32, tag="res")
        nc.vector.tensor_sub(res[:], sh[:], ln[:].broadcast_to([G, OUTC]))
        nc.sync.dma_start(out_d.ap(), res[:])

    nc.compile()
    return nc


def _wrap16(a, nsec):
    # idx e of each section at [e % 16, e // 16], replicated to all 8
    # gpsimd 16-partition groups -> [128, nsec*SECI]
    seci = a.shape[0] // nsec // 16
    t = a.reshape(nsec, seci, 16).transpose(0, 2, 1)
    t = np.tile(t, (1, 8, 1))
    return np.ascontiguousarray(t.transpose(1, 0, 2).reshape(128, nsec * seci))


def _wrap128(a, ncols):
    # edge e at [e % 128, e // 128] -> [128, ncols]
    return np.ascontiguousarray(a.reshape(ncols, 128).T)


def _prep(x, edge_index, edge_weight, batch):
    # gcn_norm with self loops, exactly like the reference (fp32)
    loop = np.arange(N, dtype=edge_index.dtype)
    src = np.concatenate([edge_index[0], loop]).astype(np.int64)
    dst = np.concatenate([edge_index[1], loop]).astype(np.int64)
    w = np.concatenate([edge_weight,
                        np.ones(N, edge_weight.dtype)]).astype(np.float32)
    deg = np.zeros(N, np.float32)
    np.add.at(deg, dst, w)
    dinv = np.where(deg > 0, 1.0 / np.sqrt(deg), 0.0).astype(np.float32)
    norm = (dinv[src] * w * dinv[dst]).astype(np.float32)

    core = dst // NPC
    srcrow = (src // NPC) * NPAD + (src % NPC)
    dstloc = dst - core * NPC
    tid = dstloc // 128          # dst tile within core
    dlocal = dstloc % 128
    is_hi = srcrow >= HALF

    # max section length over (core, tile, half) -> SEC subchunks
    key = ((core * TILES + tid) * 2 + is_hi).astype(np.int64)
    cnt = np.bincount(key, minlength=NCORES * TILES * 2)
    SEC = int(np.ceil(cnt.max() / 128))
    SECE = SEC * 128
    SUBS = 2 * SEC

    order = np.argsort(key, kind="stable")
    ks, ss, ds, ns = key[order], srcrow[order], dlocal[order], norm[order]
    starts = np.zeros(NCORES * TILES * 2 + 1, np.int64)
    starts[1:] = np.cumsum(cnt)

    in_maps = []
    eye = np.eye(128, dtype=np.float32)
    iota = np.tile(np.arange(G, dtype=np.int32), (128, 1))
    iotad = np.tile(np.arange(128, dtype=np.float32), (128, 1))
    for c in range(NCORES):
        si = np.zeros((TILES * 2, SECE), np.int16)
        df = np.full((TILES * 2, SECE), -1.0, np.float32)
        nv = np.zeros((TILES * 2, SECE), np.float32)
        for t in range(TILES):
            for h in range(2):
                k = (c * TILES + t) * 2 + h
                a, b = starts[k], starts[k + 1]
                n = b - a
                row = t * 2 + h
                si[row, :n] = (ss[a:b] - h * HALF).astype(np.int16)
                df[row, :n] = ds[a:b]
                nv[row, :n] = ns[a:b]
        # src idx: per section wrap16; dstf/normv: per tile [2*SECE] wrap128
        simap = _wrap16(si.reshape(-1), TILES * 2)
        dfmap = _wrap128(df.reshape(-1), TILES * SUBS)
        nvmap = _wrap128(nv.reshape(-1), TILES * SUBS)

        xo = np.zeros((F, NPAD), np.float32)
        xo[:, :NPC] = x[c * NPC:(c + 1) * NPC].T
        bt = np.full(NPAD, -1, np.int32)
        bt[:NPC] = batch[c * NPC:(c + 1) * NPC]
        in_maps.append({
            "x_ownT": xo,
            "src_idx": simap,
            "dstf": dfmap,
            "normv": nvmap,
            "batch_t": np.ascontiguousarray(
                bt.reshape(TILES, 128).T.astype(np.int32)),
            "eye": eye, "iota_g": iota, "iota_d": iotad,
        })
    return in_maps, SEC


def kernel(x, edge_index, edge_weight, batch, W1, b1, W2, b2, W3, b3):
    global _COMPILED
    x = np.asarray(x, np.float32)
    edge_index = np.asarray(edge_index)
    edge_weight = np.asarray(edge_weight, np.float32)
    batch = np.asarray(batch)

    in_maps, SEC = _prep(x, edge_index, edge_weight, batch)
    consts = {
        "W1": np.asarray(W1, np.float32),
        "W2": np.asarray(W2, np.float32),
        "W3": np.asarray(W3, np.float32),
        "b1rep": np.tile(np.asarray(b1, np.float32), (128, 1)),
        "b2rep": np.tile(np.asarray(b2, np.float32), (128, 1)),
        "b3rep": np.tile(np.asarray(b3, np.float32), (G, 1)),
    }
    for m in in_maps:
        m.update(consts)

    if _COMPILED is None or _COMPILED[1] != SEC:
        nc = _build_program(SEC)
        _COMPILED = (nc, SEC)
    nc = _COMPILED[0]

    from concourse.bass_utils import run_bass_kernel_spmd
    trace = os.environ.get("GNN_TRACE", "") == "1"
    res = run_bass_kernel_spmd(
        nc, in_maps, core_ids=list(range(NCORES)), trace=trace)
    if trace:
        kernel.last_exec_ns = res.exec_time_ns
        kernel.last_profile = res.profile_json
    return np.asarray(res.results[0]["out"], np.float32)



# revision 12
# speedup vs baseline: 4.3622x; 4.3622x over previous
"""3-layer GCN (message passing) on 8 Trainium2 NeuronCores.

Strategy: nodes are rebalanced (by in-degree) into 8 cores x 49 tiles of
128 so every (tile, src-chunk) edge section has near-equal size.  Per
layer each core densely transforms its node shard (PE matmul, bf16),
AllGathers the transformed table in three chunks (tiles 0-16 / 17-32 /
33-48) so later chunks' collectives overlap earlier chunks'
aggregation, then per dst tile one dma_gather per section pulls h[src]
rows (bf16, 256B descriptors) and the PE matmuls them against
norm-scaled one-hot dst masks built on DVE from static data (so mask
construction never sits on the gather critical path); operands are
swapped so the PE produces the aggregation transposed, which feeds the
next transform directly (no PE transposes anywhere).  Layer 3 is folded
on the host: pool(A @ (h2 W3)) == (pool A) @ (h2 W3), so a precomputed
C matrix turns the last aggregation + mean-pool into one matmul chain
per tile and a tiny AllReduce.  log_softmax runs on device; host work
is static graph preprocessing only.
"""

import os
import sys
from contextlib import ExitStack

import numpy as np
import ml_dtypes

sys.path.insert(0, "/opt/trn_rl_repo")

BF16 = ml_dtypes.bfloat16

# problem constants (hardcoded per the harness contract)
N = 50000
E = 800000
F = 128          # DIN == HID == 128
OUTC = 10
OUTP = 16        # padded
G = 64
NCORES = 8
TILES = 49
NPC = TILES * 128             # 6272 padded nodes per core
CHUNKS = (17, 16, 16)         # tiles per allgather chunk
CBASE = (0, 17, 33)
NCH = 3
SUBS_MAX = 8                  # sections are sized to fit one gather call
GCAP = 8                      # max subchunks (1024 descriptors) per call

_COMPILED = None  # (nc, key) cache


def _build_program(meta):
    """meta: list over TILES*NCH sections (tile-major, chunk minor) of
    dicts with SUBS, NIDX, plus cumulative column offsets ioff/soff."""
    import concourse.tile as tile
    from concourse import bacc, mybir

    f32 = mybir.dt.float32
    bf16 = mybir.dt.bfloat16
    i16 = mybir.dt.int16
    AF = mybir.ActivationFunctionType
    ALU = mybir.AluOpType

    nc = bacc.Bacc("TRN2", target_bir_lowering=False, debug=False,
                   enable_asserts=False, num_devices=NCORES,
                   num_swdge_queues=4, dynamic_dma_scratch_size=65536)

    TOTS = sum(m["SUBS"] for m in meta)         # total subchunk columns
    TOTI = sum(m["NIDX"] // 16 for m in meta)   # total idx columns

    # ---- I/O tensors ----
    xT_d = nc.dram_tensor("x_ownT", [F, NPC], bf16, kind="ExternalInput")
    W1_d = nc.dram_tensor("W1", [F, F], bf16, kind="ExternalInput")
    W2_d = nc.dram_tensor("W2", [F, F], bf16, kind="ExternalInput")
    W3_d = nc.dram_tensor("W3p", [F, OUTP], bf16, kind="ExternalInput")
    b1_d = nc.dram_tensor("b1T", [128, 128], f32, kind="ExternalInput")
    b2_d = nc.dram_tensor("b2T", [128, 128], f32, kind="ExternalInput")
    fin_d = nc.dram_tensor("fin", [G, 2 * OUTC], f32, kind="ExternalInput")
    iota_d = nc.dram_tensor("iota3", [128, SUBS_MAX * 128], bf16,
                            kind="ExternalInput")
    C_d = nc.dram_tensor("Cmat", [128, TILES * G], bf16, kind="ExternalInput")
    si_d = nc.dram_tensor("src_idx", [128, TOTI], i16, kind="ExternalInput")
    nvdf_d = nc.dram_tensor("nvdf", [128, 2 * TOTS], bf16,
                            kind="ExternalInput")
    out_d = nc.dram_tensor("out", [G, OUTC], f32, kind="ExternalOutput")

    # ---- internal DRAM ----
    T_own = [[nc.dram_tensor(f"T{l}{c}_own", [CHUNKS[c] * 128, F], bf16)
              for c in range(NCH)] for l in range(2)]
    T_full = [[nc.dram_tensor(f"T{l}{c}_full",
                              [NCORES * CHUNKS[c] * 128, F], bf16,
                              addr_space="Shared") for c in range(NCH)]
              for l in range(2)]
    pool_in = nc.dram_tensor("pool_in", [G, OUTP], f32)
    pool_out = nc.dram_tensor("pool_out", [G, OUTP], f32)
    groups = [list(range(NCORES))]

    with tile.TileContext(nc) as tc, ExitStack() as ctx:
        const = ctx.enter_context(tc.tile_pool(name="const", bufs=1))
        sb = ctx.enter_context(tc.tile_pool(name="sb", bufs=4))
        gpool = ctx.enter_context(tc.tile_pool(name="gath", bufs=4))
        mpool = ctx.enter_context(tc.tile_pool(name="mask", bufs=4))
        psum = ctx.enter_context(tc.tile_pool(name="psum", bufs=2,
                                              space="PSUM"))
        ppsum = ctx.enter_context(tc.tile_pool(name="ppsum", bufs=1,
                                               space="PSUM"))

        def load_const(tag, dram, shape, dtype):
            t = const.tile(shape, dtype, tag=tag)
            nc.sync.dma_start(t[:], dram.ap())
            return t

        W1_sb = load_const("W1", W1_d, [F, F], bf16)
        W2_sb = load_const("W2", W2_d, [F, F], bf16)
        W3_sb = load_const("W3", W3_d, [F, OUTP], bf16)
        b1_sb = load_const("b1", b1_d, [128, 128], f32)
        b2_sb = load_const("b2", b2_d, [128, 128], f32)
        fin_sb = load_const("fin", fin_d, [G, 2 * OUTC], f32)
        iota3_sb = load_const("iota3", iota_d, [128, SUBS_MAX * 128], bf16)
        C_sb = load_const("C", C_d, [128, TILES * G], bf16)
        xT_sb = load_const("xT", xT_d, [F, NPC], bf16)
        si_sb = load_const("si", si_d, [128, TOTI], i16)
        nvdf_sb = load_const("nvdf", nvdf_d, [128, 2 * TOTS], bf16)
        iota3_v = iota3_sb[:].rearrange("p (s d) -> p s d", d=128)

        def rng(t):
            return slice(t * 128, (t + 1) * 128)

        def store_own(l, t, tt):
            for c in range(NCH - 1, -1, -1):
                if t >= CBASE[c]:
                    nc.sync.dma_start(
                        T_own[l][c].ap()[rng(t - CBASE[c]), :], tt[:])
                    break

        def fire_ag(l, t):
            for c in range(NCH):
                if t == CBASE[c] + CHUNKS[c] - 1:
                    nc.gpsimd.collective_compute(
                        "AllGather", ALU.bypass, replica_groups=groups,
                        ins=[T_own[l][c].ap().opt()],
                        outs=[T_full[l][c].ap().opt()])

        # ---- phase A: T0 = x_own @ W1 (bf16), chunked allgather ----
        for t in range(TILES):
            ps = psum.tile([128, F], f32, tag="tps")
            nc.tensor.matmul(ps[:], xT_sb[:, rng(t)], W1_sb[:],
                             start=True, stop=True)
            tt = sb.tile([128, F], bf16, tag="tout")
            nc.scalar.activation(tt[:], ps[:], AF.Copy)
            store_own(0, t, tt)
            fire_ag(0, t)

        # ---- aggregation of one dst tile into a PSUM tile (transposed:
        #      psT[F, dst] += sum_s g[e,:]^T @ mask[e,dst]) ----
        def agg_tile(l, t, bias_sb):
            ps = psum.tile([128, 128], f32, tag="aps")
            nc.vector.tensor_copy(ps[:], bias_sb[:])
            secs = [meta[NCH * t + h] for h in range(NCH)]
            hlast = max((h for h in range(NCH) if secs[h]["SUBS"]),
                        default=-1)
            for h in range(NCH):
                m = secs[h]
                SUBS, NIDX = m["SUBS"], m["NIDX"]
                if SUBS == 0:
                    continue
                # mask: nv-scaled one-hot over dst slots (static inputs
                # only -- runs ahead of the gather)
                so = 2 * m["soff"]
                mk = mpool.tile([128, SUBS, 128], bf16, tag=f"mk{h}")
                nc.vector.tensor_tensor(
                    mk[:],
                    nvdf_sb[:, so + SUBS:so + 2 * SUBS].unsqueeze(2)
                    .broadcast_to([128, SUBS, 128]),
                    iota3_v[:, 0:SUBS, :],
                    op=ALU.is_equal)
                nc.vector.tensor_mul(
                    mk[:], mk[:],
                    nvdf_sb[:, so:so + SUBS].unsqueeze(2)
                    .broadcast_to([128, SUBS, 128]))
                g = gpool.tile([128, SUBS, F], bf16, tag=f"g{h}")
                for p in range(0, SUBS, GCAP):
                    take = min(GCAP, SUBS - p)
                    nidx = take * 128
                    off = m["ioff"] + p * 8
                    nc.gpsimd.dma_gather(
                        g[:, p:p + take, :], T_full[l][h].ap(),
                        si_sb[:, off:off + nidx // 16],
                        nidx, nidx, F,
                        queue_num=(NCH * t + h) % 4)
                for s in range(SUBS):
                    nc.tensor.matmul(ps[:], g[:, s, :], mk[:, s, :],
                                     start=False,
                                     stop=(h == hlast and s == SUBS - 1),
                                     skip_group_check=True)
            return ps

        # ---- layer 1: T1 = (relu(aggT + b1)) @ W2, chunked AG ----
        for t in range(TILES):
            ps = agg_tile(0, t, b1_sb)
            hT = sb.tile([128, 128], bf16, tag="h")
            nc.scalar.activation(hT[:], ps[:], AF.Relu)
            ps2 = psum.tile([128, F], f32, tag="tps")
            nc.tensor.matmul(ps2[:], hT[:], W2_sb[:], start=True, stop=True)
            tt = sb.tile([128, F], bf16, tag="tout")
            nc.scalar.activation(tt[:], ps2[:], AF.Copy)
            store_own(1, t, tt)
            fire_ag(1, t)

        # ---- layer 2 + folded layer-3 transform + pooling ----
        pp = ppsum.tile([G, OUTP], f32, tag="pp")
        for t in range(TILES):
            ps = agg_tile(1, t, b2_sb)
            hT = sb.tile([128, 128], bf16, tag="h")
            nc.scalar.activation(hT[:], ps[:], AF.Relu)
            ps2 = psum.tile([128, OUTP], f32, tag="t2ps")
            nc.tensor.matmul(ps2[:], hT[:], W3_sb[:], start=True, stop=True)
            t2 = sb.tile([128, OUTP], bf16, tag="t2")
            nc.scalar.activation(t2[:], ps2[:], AF.Copy)
            nc.tensor.matmul(pp[:], C_sb[:, t * G:(t + 1) * G], t2[:],
                             start=(t == 0), stop=(t == TILES - 1),
                             skip_group_check=True)

        pc = sb.tile([G, OUTP], f32, tag="pc")
        nc.scalar.activation(pc[:], pp[:], AF.Copy)
        nc.sync.dma_start(pool_in.ap(), pc[:])
        nc.gpsimd.collective_compute(
            "AllReduce", ALU.add, replica_groups=groups,
            ins=[pool_in.ap().opt()], outs=[pool_out.ap().opt()])
        pr = sb.tile([G, OUTP], f32, tag="pr")
        nc.sync.dma_start(pr[:], pool_out.ap())

        # m = pr[:, :OUTC] * rec + b3 ; log_softmax
        m = sb.tile([G, OUTC], f32, tag="m")
        nc.vector.tensor_mul(m[:], pr[:, 0:OUTC], fin_sb[:, 0:OUTC])
        nc.vector.tensor_add(m[:], m[:], fin_sb[:, OUTC:2 * OUTC])
        mx = sb.tile([G, 1], f32, tag="mx")
        nc.vector.tensor_reduce(mx[:], m[:], mybir.AxisListType.XYZW, ALU.max)
        sh = sb.tile([G, OUTC], f32, tag="sh")
        nc.vector.tensor_sub(sh[:], m[:], mx[:].broadcast_to([G, OUTC]))
        ex = sb.tile([G, OUTC], f32, tag="ex")
        nc.scalar.activation(ex[:], sh[:], AF.Exp)
        sm = sb.tile([G, 1], f32, tag="sm")
        nc.vector.tensor_reduce(sm[:], ex[:], mybir.AxisListType.XYZW, ALU.add)
        ln = sb.tile([G, 1], f32, tag="ln")
        nc.scalar.activation(ln[:], sm[:], AF.Ln)
        res = sb.tile([G, OUTC], f32, tag="res")
        nc.vector.tensor_sub(res[:], sh[:], ln[:].broadcast_to([G, OUTC]))
        nc.sync.dma_start(out_d.ap(), res[:])

    nc.compile()
    return nc


def _wrap16(a):
    # idx e at [e % 16, e // 16], replicated to all 8 gpsimd groups
    cols = a.shape[0] // 16
    t = a.reshape(cols, 16).T
    return np.ascontiguousarray(np.tile(t, (8, 1)))


def _wrap128(a):
    # edge e at [e % 128, e // 128]
    return np.ascontiguousarray(a.reshape(-1, 128).T)


def _prep(x, edge_index, edge_weight, batch):
    # gcn_norm with self loops, exactly like the reference (fp32)
    loop = np.arange(N, dtype=np.int64)
    src = np.concatenate([edge_index[0].astype(np.int64), loop])
    dst = np.concatenate([edge_index[1].astype(np.int64), loop])
    w = np.concatenate([edge_weight,
                        np.ones(N, edge_weight.dtype)]).astype(np.float32)
    deg = np.zeros(N, np.float32)
    np.add.at(deg, dst, w)
    dinv = np.where(deg > 0, 1.0 / np.sqrt(deg), 0.0).astype(np.float32)
    norm = (dinv[src] * w * dinv[dst]).astype(np.float32)

    # ---- balance nodes into 8*49 buckets by in-edge count (snake) ----
    NB = NCORES * TILES
    cnt_in = np.bincount(dst, minlength=N).astype(np.int64)
    order = np.argsort(-cnt_in, kind="stable")
    core_a = np.empty(N, np.int64)
    tile_a = np.empty(N, np.int64)
    slot_a = np.empty(N, np.int64)
    fwd = np.arange(NB)
    rev = fwd[::-1]
    pos = 0
    r = 0
    while pos < N:
        take = min(NB, N - pos)
        buckets = (fwd if r % 2 == 0 else rev)[:take]
        nodes = order[pos:pos + take]
        core_a[nodes] = buckets % NCORES
        tile_a[nodes] = buckets // NCORES
        slot_a[nodes] = r
        pos += take
        r += 1

    # chunk of a node's tile + row id within that chunk's gathered table
    cof = np.zeros(TILES, np.int64)
    for c in range(NCH):
        cof[CBASE[c]:CBASE[c] + CHUNKS[c]] = c
    chunk_n = cof[tile_a]
    csz = np.array([CHUNKS[c] * 128 for c in range(NCH)], np.int64)
    cbase = np.array(CBASE, np.int64)
    rowid = (core_a * csz[chunk_n] + (tile_a - cbase[chunk_n]) * 128 +
             slot_a)

    # ---- edge sections keyed by (dst core, dst tile, src chunk) ----
    key = (core_a[dst] * TILES + tile_a[dst]) * NCH + chunk_n[src]
    cnt = np.bincount(key, minlength=NB * NCH)
    order_e = np.argsort(key, kind="stable")
    ss = rowid[src][order_e].astype(np.int16)
    ds_ = slot_a[dst][order_e].astype(np.float32)
    ns = norm[order_e]
    starts = np.zeros(NB * NCH + 1, np.int64)
    starts[1:] = np.cumsum(cnt)

    # per (tile, chunk): max count over cores -> SUBS / NIDX
    cnt_tc = cnt.reshape(NCORES, TILES, NCH)
    nmax = cnt_tc.max(axis=0)                      # [TILES, NCH]
    meta = []
    ioff = soff = 0
    for t in range(TILES):
        for h in range(NCH):
            n = int(nmax[t, h])
            SUBS = -(-n // 128)
            NIDX = SUBS * 128
            meta.append({"SUBS": SUBS, "NIDX": NIDX,
                         "ioff": ioff, "soff": soff})
            ioff += NIDX // 16
            soff += SUBS

    TOTI = ioff
    TOTS = soff
    in_maps = []
    iota3 = np.tile(np.arange(128, dtype=np.float32),
                    (128, SUBS_MAX)).astype(BF16)
    for c in range(NCORES):
        si = np.zeros(TOTI * 16, np.int16)
        nv = np.zeros(TOTS * 128, np.float32)
        df = np.full(TOTS * 128, -1.0, np.float32)
        for t in range(TILES):
            for h in range(NCH):
                mm = meta[NCH * t + h]
                k = (c * TILES + t) * NCH + h
                a, b = starts[k], starts[k + 1]
                n = b - a
                si[mm["ioff"] * 16:mm["ioff"] * 16 + n] = ss[a:b]
                nv[mm["soff"] * 128:mm["soff"] * 128 + n] = ns[a:b]
                df[mm["soff"] * 128:mm["soff"] * 128 + n] = ds_[a:b]
        simap = _wrap16(si)
        nvdf = np.zeros((128, 2 * TOTS), np.float32)
        nvw = _wrap128(nv)
        dfw = _wrap128(df)
        for mm in meta:
            S, o = mm["SUBS"], mm["soff"]
            nvdf[:, 2 * o:2 * o + S] = nvw[:, o:o + S]
            nvdf[:, 2 * o + S:2 * o + 2 * S] = dfw[:, o:o + S]

        mine = core_a == c
        lrow = (tile_a * 128 + slot_a)[mine]
        xo = np.zeros((F, NPC), np.float32)
        xo[:, lrow] = x[mine].T
        in_maps.append({
            "x_ownT": xo.astype(BF16),
            "src_idx": simap,
            "nvdf": nvdf.astype(BF16),
            "iota3": iota3,
        })

    # ---- folded pooling matrix C: [slot, tile*G + g] per core ----
    # pooled_sum[g] = sum_e norm_e * [batch[dst_e]==g] * t2'[src_e]
    csrc = core_a[src]
    lsrc = tile_a[src] * 128 + slot_a[src]
    gdst = batch[dst].astype(np.int64)
    for c in range(NCORES):
        Cd = np.zeros((NPC, G), np.float32)
        m_ = csrc == c
        np.add.at(Cd, (lsrc[m_], gdst[m_]), norm[m_])
        Cm = np.zeros((128, TILES * G), np.float32)
        for t in range(TILES):
            Cm[:, t * G:(t + 1) * G] = Cd[t * 128:(t + 1) * 128, :]
        in_maps[c]["Cmat"] = Cm.astype(BF16)

    key_t = tuple((m["SUBS"], m["NIDX"]) for m in meta)
    return in_maps, meta, key_t


def kernel(x, edge_index, edge_weight, batch, W1, b1, W2, b2, W3, b3):
    global _COMPILED
    x = np.asarray(x, np.float32)
    edge_index = np.asarray(edge_index)
    edge_weight = np.asarray(edge_weight, np.float32)
    batch = np.asarray(batch)

    in_maps, meta, key_t = _prep(x, edge_index, edge_weight, batch)

    cnts = np.bincount(batch, minlength=G).astype(np.float32)
    rec = 1.0 / np.maximum(cnts, 1.0)
    fin = np.zeros((G, 2 * OUTC), np.float32)
    fin[:, 0:OUTC] = rec[:, None]
    fin[:, OUTC:2 * OUTC] = np.asarray(b3, np.float32)[None, :]
    W3p = np.zeros((F, OUTP), np.float32)
    W3p[:, 0:OUTC] = np.asarray(W3, np.float32)
    consts = {
        "W1": np.asarray(W1, np.float32).astype(BF16),
        "W2": np.asarray(W2, np.float32).astype(BF16),
        "W3p": W3p.astype(BF16),
        "b1T": np.tile(np.asarray(b1, np.float32).reshape(128, 1), (1, 128)),
        "b2T": np.tile(np.asarray(b2, np.float32).reshape(128, 1), (1, 128)),
        "fin": fin,
    }
    for m in in_maps:
        m.update(consts)

    if _COMPILED is None or _COMPILED[1] != key_t:
        nc = _build_program(meta)
        _COMPILED = (nc, key_t)
    nc = _COMPILED[0]

    from concourse.bass_utils import run_bass_kernel_spmd
    trace = os.environ.get("GNN_TRACE", "") == "1"
    res = run_bass_kernel_spmd(
        nc, in_maps, core_ids=list(range(NCORES)), trace=trace)
    if trace:
        kernel.last_exec_ns = res.exec_time_ns
        kernel.last_profile = res.profile_json
    return np.asarray(res.results[0]["out"], np.float32)
